# revision 29
# baseline (speedup 1.0000x reference)
"""Gated multi-head attention (RMSNorm + RoPE + SDPA + sigmoid head gates + out-proj)
as a Trainium2 Bass/Tile kernel, data-parallel over batch across 8 NeuronCores.

Problem shapes (hardcoded): b=8, n=1024, dim=512, heads=8, dim_head=64, theta=1e4.
Each core processes one batch element; no collectives needed.

v3: software-pipelined single PE stream, float32r projections.

The scalar engine's exp of the n^2 score matrix (64 tiles of [128,1024],
~80us) is the pacing resource. The kernel runs 4 pair-windows of 8 "beats"
(beat = S^T tiles for one key chunk, both heads, + their exps); all other PE
work — v/qk projections of the next pair, the previous pair's AV passes and
gate broadcast, out-projection — is emitted as beat fillers so the tensor
engine stays dense (HAM p-state) and no other engine exceeds the exp budget.

float32r: q/k/v/gate projections consume the fp32 weight stages and the fp32
transposed activations directly (1 cycle/row at >=256 free size), so there
are no weight fold-casts at all; gamma folds into the transpose drains.
Scores/AV stay bf16 (qT/kT/es/v_aug) for SBUF capacity. The AV matmul's
ones-columns produce the softmax denominator on the spare partition half.

Engine roles:
  ACT: RMS square/sqrt, half the transpose drains (Copy*gamma), gates
       exp (sigmoid via 1/(1+e^-z): stays on the Exp act table), 64x exp,
       tail out-copies
  DVE: RMS recip+scale, other drains, RoPE shuffle/mul/add, v_aug + avraw
       psum drains, denominator recip, gate scale
  Pool(gpsimd): cos/sin/wg/wo/identity SWDGE loads, w_o cast, RoPE sin mul,
       v_aug ones memset (gpsimd never touches PSUM)
  SP:  x/w_qkv/gamma loads, denominator row moves, output stores
PSUM: 2x[128,1024] score tiles (4 banks; also transpose staging) +
4x[128,512] utility tiles (4 banks) shared by v/qk/gates/AV/cb/out-proj.
"""

import sys

if "/opt/trn_rl_repo" not in sys.path:
    sys.path.insert(0, "/opt/trn_rl_repo")

import numpy as np

import concourse.bass as bass
import concourse.tile as tile
from concourse import bacc, mybir
from concourse.bass_utils import run_bass_kernel_spmd

F32 = mybir.dt.float32
F32R = mybir.dt.float32r
BF16 = mybir.dt.bfloat16
AF = mybir.ActivationFunctionType
ALU = mybir.AluOpType

B = 8
N = 1024
DIM = 512
HEADS = 8
DHEAD = 64
THETA = 10000.0
N_CORES = 8

NT = N // 128  # 8 token row tiles
KO = DIM // 128  # 4 contraction chunks
NC_ = N // 512  # 2 query column chunks of 512
MT = 4  # head pairs (2 heads x 64 dims = 128 partitions)
JC = 8  # key chunks of 128


def _rope_tables():
    """cos2T/sinS2T [128, N]: rows p = (h%2)*64 + d; identical per head half.

    sinS2T row 2t   = -sin(n * invf[t])  (multiplies shuffled value q[2t+1])
    sinS2T row 2t+1 = +sin(n * invf[t])
    """
    inv_freq = 1.0 / (THETA ** (np.arange(0, DHEAD, 2, dtype=np.float64) / DHEAD))
    pos = np.arange(N, dtype=np.float64)
    freqs = pos[None, :] * np.repeat(inv_freq, 2)[:, None]  # [64, N]
    cos = np.cos(freqs)
    sin = np.sin(freqs)
    sign = np.where(np.arange(DHEAD) % 2 == 0, -1.0, 1.0)[:, None]
    sin_signed = sin * sign
    cos2 = np.concatenate([cos, cos], axis=0).astype(np.float32)
    sin2 = np.concatenate([sin_signed, sin_signed], axis=0).astype(np.float32)
    return np.ascontiguousarray(cos2), np.ascontiguousarray(sin2)


def build_kernel():
    import ml_dtypes

    nc = bacc.Bacc("TRN2", target_bir_lowering=False, debug=False, num_devices=N_CORES)

    x_d = nc.dram_tensor("x", [N, DIM], F32, kind="ExternalInput").ap()
    gamma_d = nc.dram_tensor("gamma", [DIM], F32, kind="ExternalInput").ap()
    wqkv_d = nc.dram_tensor("w_qkv", [DIM, 3 * DIM], F32, kind="ExternalInput").ap()
    wg_d = nc.dram_tensor("w_g", [DIM, HEADS], F32, kind="ExternalInput").ap()
    bg_d = nc.dram_tensor("b_g", [HEADS], F32, kind="ExternalInput").ap()
    wo_d = nc.dram_tensor("w_o", [DIM, DIM], F32, kind="ExternalInput").ap()
    out_d = nc.dram_tensor("out", [N, DIM], F32, kind="ExternalOutput").ap()

    cos_np, sin_np = _rope_tables()
    cos_d = nc.inline_tensor(cos_np.astype(ml_dtypes.bfloat16), name="rope_cos").ap()
    sin_d = nc.inline_tensor(sin_np.astype(ml_dtypes.bfloat16), name="rope_sin").ap()
    eye_d = nc.inline_tensor(np.eye(128, dtype=np.float32), name="eye128").ap()

    # E[h, mt*128 + p] = 1 if head h owns partition p of pair-tile mt
    e_np = np.zeros((HEADS, MT * 128), np.float32)
    for mt in range(MT):
        for p in range(128):
            e_np[2 * mt + p // 64, mt * 128 + p] = 1.0
    e_d = nc.inline_tensor(e_np.astype(ml_dtypes.bfloat16), name="gate_bcast_e").ap()

    with tile.TileContext(nc) as tc:
        _build_tile(
            nc, tc, x_d, gamma_d, wqkv_d, wg_d, bg_d, wo_d, cos_d, sin_d, eye_d, e_d, out_d
        )

    nc.compile()
    return nc


def _build_tile(
    nc, tc, x_d, gamma_d, wqkv_d, wg_d, bg_d, wo_d, cos_d, sin_d, eye_d, e_d, out_d
):
    from contextlib import ExitStack

    ctx = ExitStack()
    with ctx:
        singles = ctx.enter_context(tc.tile_pool(name="singles", bufs=1))
        wpool = ctx.enter_context(tc.tile_pool(name="weights", bufs=1))
        wstage = ctx.enter_context(tc.tile_pool(name="wstage", bufs=1))
        xpool = ctx.enter_context(tc.tile_pool(name="x", bufs=1))
        xtbp = ctx.enter_context(tc.tile_pool(name="xtb", bufs=1))
        xtp = ctx.enter_context(tc.tile_pool(name="xhatT", bufs=1))
        qkpool = ctx.enter_context(tc.tile_pool(name="qk", bufs=1))
        vpool = ctx.enter_context(tc.tile_pool(name="vaug", bufs=1))
        spool = ctx.enter_context(tc.tile_pool(name="expS", bufs=24))
        gpool = ctx.enter_context(tc.tile_pool(name="gates", bufs=1))
        avpool = ctx.enter_context(tc.tile_pool(name="avg", bufs=1))
        scratch = ctx.enter_context(tc.tile_pool(name="scratch", bufs=2))

        # ---- x + gamma + w_qkv on the SP ring ----
        xts = [
            xpool.tile([128, DIM], F32, tag=f"xt{it % 6}", name=f"xt{it}")
            for it in range(NT)
        ]
        for it in (0, 1, 2, 3):
            nc.sync.dma_start(out=xts[it][:], in_=x_d[it * 128:(it + 1) * 128, :])
        gamma_sb = singles.tile([128, KO], F32)
        nc.sync.dma_start(
            out=gamma_sb[:], in_=gamma_d.rearrange("(ko ki) -> ki ko", ki=128)
        )
        bg_sb = singles.tile([HEADS, 1], F32)
        nc.sync.dma_start(out=bg_sb[:], in_=bg_d.rearrange("(h o) -> h o", o=1))
        def _w_stage(ko):
            return wstage.tile(
                [128, 3 * DIM], F32, tag=f"ws{ko % 2}", name=f"ws{ko}", bufs=1
            )

        def _w_casts(ko, ws):
            # q|k columns on whichever engine is free soonest (Pool is idle
            # during the RMS phase; ACT/DVE pick up the rest right after);
            # v columns on Pool afterwards (first needed a window later)
            if ko < 2:
                nc.gpsimd.tensor_copy(
                    out=wqkv_bf[:, ko, 0:2 * DIM], in_=ws[:, 0:2 * DIM]
                )
            elif ko == 2:
                nc.scalar.copy(out=wqkv_bf[:, ko, 0:2 * DIM], in_=ws[:, 0:2 * DIM])
            else:
                nc.vector.tensor_copy(
                    out=wqkv_bf[:, ko, 0:2 * DIM], in_=ws[:, 0:2 * DIM]
                )

        def _w_casts_v(ko, ws):
            nc.gpsimd.tensor_copy(
                out=wqkv_bf[:, ko, 2 * DIM:3 * DIM], in_=ws[:, 2 * DIM:3 * DIM]
            )

        wss = [_w_stage(0), _w_stage(1)]
        nc.sync.dma_start(out=wss[0][:], in_=wqkv_d[0:128, :])
        nc.sync.dma_start(out=wss[1][:], in_=wqkv_d[128:256, :])
        for it in (4, 5, 6, 7):
            nc.sync.dma_start(out=xts[it][:], in_=x_d[it * 128:(it + 1) * 128, :])
        # Preload the Sqrt act-table during DMA dead time: Square (every
        # table) then the real Sqrts would otherwise force a reload mid-RMS.
        warmt = scratch.tile([1, 1], F32, tag="warmt", name="warmt", bufs=1)
        nc.gpsimd.memset(warmt[:], 1.0)
        nc.scalar.activation(out=warmt[:], in_=warmt[:], func=AF.Sqrt)

        # ---- small/constant loads on the Pool SWDGE ring ----
        ident = singles.tile([128, 128], BF16)
        nc.gpsimd.dma_start(out=ident[:], in_=eye_d[:])
        cosT = singles.tile([128, N], BF16)
        sinT = singles.tile([128, N], BF16)
        nc.gpsimd.dma_start(out=cosT[:], in_=cos_d[:])
        nc.gpsimd.dma_start(out=sinT[:], in_=sin_d[:])
        wg_s = wpool.tile([128, KO, HEADS], F32)
        nc.gpsimd.dma_start(
            out=wg_s[:], in_=wg_d.rearrange("(ko ki) h -> ki ko h", ki=128)
        )
        wg_bf = wpool.tile([128, KO, HEADS], BF16)
        nc.gpsimd.tensor_copy(out=wg_bf[:], in_=wg_s[:])
        e_sb = singles.tile([HEADS, MT * 128], BF16, name="e_sb")
        nc.gpsimd.dma_start(out=e_sb[:], in_=e_d[:])
        wo_st = wpool.tile([128, KO, DIM], F32)
        nc.gpsimd.dma_start(
            out=wo_st[:], in_=wo_d.rearrange("(ko ki) d -> ki ko d", ki=128)
        )
        wo_sb = wpool.tile([128, KO, DIM], BF16)
        for ko in range(KO):
            nc.gpsimd.tensor_copy(out=wo_sb[:, ko, :], in_=wo_st[:, ko, :])



        # ---- RMSnorm -> xtb (f32), 1-deep pipelined on ACT ----
        xhatT = [
            xtp.tile([128, N], BF16, tag=f"xhatT{ko}", name=f"xhatT{ko}")
            for ko in range(KO)
        ]
        xtb = [
            xtbp.tile([128, DIM], BF16, tag=f"xtb{it}", name=f"xtb{it}")
            for it in range(NT)
        ]
        sss = []

        def _rms_head(it):
            ss = scratch.tile([128, 1], F32, tag=f"ss{it}", name=f"ss{it}", bufs=1)
            # Square's elementwise output is unused; scribble it into xtb[it],
            # which the tensor_scalar_mul in _rms_tail overwrites.
            nc.scalar.activation(
                out=xtb[it][:], in_=xts[it][:], func=AF.Square, accum_out=ss[:]
            )
            sss.append(ss)

        def _rms_tail(it):
            nc.scalar.activation(
                out=sss[it][:], in_=sss[it][:], func=AF.Sqrt, scale=1.0 / DIM
            )
            nc.vector.reciprocal(out=sss[it][:], in_=sss[it][:])
            nc.vector.tensor_scalar_mul(
                out=xtb[it][:], in0=xts[it][:], scalar1=sss[it][:]
            )

        _rms_head(0)
        for it in range(1, NT):
            _rms_head(it)
            _rms_tail(it - 1)
        _rms_tail(NT - 1)

        negbg_sb = singles.tile([HEADS, 1], F32)
        nc.vector.tensor_scalar_mul(out=negbg_sb[:], in0=bg_sb[:], scalar1=-1.0)
        wqkv_bf = wpool.tile([128, KO, 3 * DIM], BF16)
        _w_casts(0, wss[0])
        _w_casts(1, wss[1])
        _w_casts_v(0, wss[0])
        _w_casts_v(1, wss[1])
        wss.append(_w_stage(2))
        nc.sync.dma_start(out=wss[2][:], in_=wqkv_d[256:384, :])
        _w_casts(2, wss[2])
        wss.append(_w_stage(3))
        nc.sync.dma_start(out=wss[3][:], in_=wqkv_d[384:512, :])
        _w_casts(3, wss[3])
        _w_casts_v(2, wss[2])
        _w_casts_v(3, wss[3])

        # ---- persistent SBUF for attention ----
        v_aug = [
            vpool.tile([128, HEADS * 128], BF16, tag=f"va{it}", name=f"va{it}")
            for it in range(NT)
        ]
        for it in range(NT):
            va3 = v_aug[it][:].rearrange("p (q c) -> p q c", q=HEADS // 2)
            nc.gpsimd.memset(va3[:, :, 64:192], 1.0)
        qT = [qkpool.tile([128, N], BF16, tag=f"q{mt}", name=f"q{mt}") for mt in range(MT)]
        kT = [qkpool.tile([128, N], BF16, tag=f"k{mt}", name=f"k{mt}") for mt in range(MT)]
        avg = [
            avpool.tile([128, N], BF16, tag=f"avg{mt}", name=f"avg{mt}")
            for mt in range(MT)
        ]
        gT = gpool.tile([HEADS, N], F32)
        denomW = gpool.tile([HEADS, N], F32)
        nc.gpsimd.memset(denomW[:], 1.0)
        cT = gpool.tile([HEADS, N], BF16)

        shuf_mask = [(i ^ 1) for i in range(32)]
        scale = 1.0 / float(np.sqrt(DHEAD))

        es = [[[None for _ in range(2)] for _ in range(JC)] for _ in range(MT)]
        avraw = {}

        with (
            tc.tile_pool(name="ps_s", bufs=2, space="PSUM") as ps_s,
            tc.tile_pool(name="ps_u", bufs=4, space="PSUM") as ps_u,
        ):
            def mm(out_ps, lhsT, rhs, start, stop):
                nc.tensor.matmul(out_ps, lhsT, rhs, start=start, stop=stop)

            def u_tile(nm):
                return ps_u.tile([128, 512], F32, tag="u", name=nm)

            # ---- PE transpose of xtb -> xhatT through the S psum banks
            # (bf16 view of the f32 tiles); gamma folds into the drains ----
            for ic in range(NC_):
                trp = ps_s.tile([128, N], F32, tag="sps", name=f"tr{ic}")
                trv = trp[:].bitcast(BF16).rearrange("p (ko c) -> p ko c", ko=KO)
                for s in range(4):
                    it = ic * 4 + s
                    for ko in range(KO):
                        nc.tensor.transpose(
                            trv[:, ko, s * 128:(s + 1) * 128],
                            xtb[it][:, ko * 128:(ko + 1) * 128],
                            ident[:],
                        )
                for ko in range(KO):
                    dst = xhatT[ko][:, ic * 512:(ic + 1) * 512]
                    if ko % 2 == 0:
                        nc.scalar.activation(
                            out=dst,
                            in_=trv[:, ko, :],
                            func=AF.Copy,
                            scale=gamma_sb[:, ko:ko + 1],
                        )
                    else:
                        nc.vector.tensor_scalar_mul(
                            out=dst,
                            in0=trv[:, ko, :],
                            scalar1=gamma_sb[:, ko:ko + 1],
                        )

            def gates_block():
                # sigmoid(z) = 1/(1+exp(-z)) — keeps ACT on the Exp table set
                for ic in range(NC_):
                    g_ps = u_tile("gps")
                    for ko in range(KO):
                        mm(
                            g_ps[0:HEADS, :],
                            wg_bf[:, ko, :],
                            xhatT[ko][:, ic * 512:(ic + 1) * 512],
                            start=(ko == 0),
                            stop=(ko == KO - 1),
                        )
                    isl = slice(ic * 512, (ic + 1) * 512)
                    ge = scratch.tile([HEADS, 512], F32, tag="ge", name="ge", bufs=1)
                    nc.scalar.activation(
                        out=ge[:],
                        in_=g_ps[0:HEADS, :],
                        func=AF.Exp,
                        scale=-1.0,
                        bias=negbg_sb[:],
                    )
                    nc.vector.tensor_scalar_add(out=ge[:], in0=ge[:], scalar1=1.0)
                    nc.vector.reciprocal_approx_fast(out=gT[:, isl], in_=ge[:])

            def v_block(it):
                """Project v row-tile it; drain into v_aug pair blocks."""
                vps = u_tile("vps")
                for ko in range(KO):
                    mm(
                        vps[:],
                        xhatT[ko][:, it * 128:(it + 1) * 128],
                        wqkv_bf[:, ko, 2 * DIM:3 * DIM],
                        start=(ko == 0),
                        stop=(ko == KO - 1),
                    )
                # src col q*128+s*64+d -> dst col q*256 + s*192 + d
                vsrc = vps[:].rearrange("p (q s d) -> p q s d", q=4, s=2)
                vdst = v_aug[it][:].rearrange("p (q c) -> p q c", q=4)
                nc.vector.tensor_copy(out=vdst[:, :, 0:64], in_=vsrc[:, :, 0, :])
                nc.vector.tensor_copy(out=vdst[:, :, 192:256], in_=vsrc[:, :, 1, :])

            def qk_block(mt, which, ic):
                """Project+RoPE one [128, 512] chunk of qT/kT for pair mt."""
                dest = qT if which == 0 else kT
                col0 = which * DIM + mt * 128
                nslice = slice(ic * 512, (ic + 1) * 512)
                pps = u_tile("pps")
                for ko in range(KO):
                    mm(
                        pps[:],
                        wqkv_bf[:, ko, col0:col0 + 128],
                        xhatT[ko][:, nslice],
                        start=(ko == 0),
                        stop=(ko == KO - 1),
                    )
                shuf = scratch.tile([128, 512], F32, tag="shuf", name="shuf", bufs=2)
                nc.vector.stream_shuffle(shuf[:], pps[:], mask=shuf_mask)
                t1 = scratch.tile([128, 512], F32, tag="rt1", name="rt1", bufs=2)
                nc.vector.tensor_tensor(
                    out=t1[:], in0=pps[:], in1=cosT[:, nslice], op=ALU.mult
                )
                t2 = scratch.tile([128, 512], F32, tag="rt2", name="rt2", bufs=2)
                nc.gpsimd.tensor_tensor(
                    out=t2[:], in0=shuf[:], in1=sinT[:, nslice], op=ALU.mult
                )
                nc.vector.tensor_tensor(
                    out=dest[mt][:, nslice], in0=t1[:], in1=t2[:], op=ALU.add
                )

            def s_block(mt, jc, hh):
                """Scores S^T [128 keys, N queries] for (pair, key chunk, head) + exp."""
                pr = slice(hh * 64, (hh + 1) * 64)
                sp = ps_s.tile([128, N], F32, tag="sps", name="sps")
                for ic in range(NC_):
                    mm(
                        sp[:, ic * 512:(ic + 1) * 512],
                        kT[mt][pr, jc * 128:(jc + 1) * 128],
                        qT[mt][pr, ic * 512:(ic + 1) * 512],
                        start=True,
                        stop=True,
                    )
                e = spool.tile([128, N], BF16, tag="es", name="es")
                nc.scalar.activation(out=e[:], in_=sp[:], func=AF.Exp, scale=scale)
                es[mt][jc][hh] = e

            AV_ORDER = ((0, 0), (1, 0), (0, 1), (1, 1))  # (ic, hh)

            def av_pass(mt, p, drain_act=False):
                """One AV accumulation pass (ic, hh) over all key chunks."""
                ic, hh = AV_ORDER[p]
                h = 2 * mt + hh
                base = (h // 2) * 256 + (h % 2) * 128
                if mt not in avraw:
                    avraw[mt] = avpool.tile(
                        [128, 2 * N], F32, tag="avraw", name=f"avraw{mt}", bufs=2
                    )
                avp = u_tile(f"avp{p}")
                for jc in range(JC):
                    mm(
                        avp[:],
                        v_aug[jc][:, base:base + 128],
                        es[mt][jc][hh][:, ic * 512:(ic + 1) * 512],
                        start=(jc == 0),
                        stop=(jc == JC - 1),
                    )
                dst = avraw[mt][:, hh * N + ic * 512:hh * N + (ic + 1) * 512]
                if drain_act:
                    nc.scalar.copy(out=dst, in_=avp[:])
                else:
                    nc.vector.tensor_copy(out=dst, in_=avp[:])

            def denom_dmas(mt, ic):
                h0 = 2 * mt
                nc.sync.dma_start(
                    out=denomW[h0:h0 + 1, ic * 512:(ic + 1) * 512],
                    in_=avraw[mt][64:65, ic * 512:(ic + 1) * 512],
                )
                nc.sync.dma_start(
                    out=denomW[h0 + 1:h0 + 2, ic * 512:(ic + 1) * 512],
                    in_=avraw[mt][0:1, N + ic * 512:N + (ic + 1) * 512],
                )

            def gate_recip(mt, ic):
                # DVE partition bases must be 32-aligned: process all 8 head
                # rows (cost is free-size-bound); stale rows are finite (the
                # denomW memset) and meet zero E-columns in cb_avg.
                isl = slice(ic * 512, (ic + 1) * 512)
                dscr = scratch.tile([HEADS, 512], F32, tag="dscr", name="dscr", bufs=1)
                nc.vector.reciprocal_approx_fast(out=dscr[:], in_=denomW[:, isl])
                nc.vector.tensor_tensor(
                    out=cT[:, isl], in0=dscr[:], in1=gT[:, isl], op=ALU.mult
                )

            def cb_avg(mt, ic):
                """Broadcast gate/denominator row to pair partitions; scale AV."""
                cb_ps = u_tile("cbps")
                mm(
                    cb_ps[:],
                    e_sb[:, mt * 128:(mt + 1) * 128],
                    cT[:, ic * 512:(ic + 1) * 512],
                    start=True,
                    stop=True,
                )
                isl = slice(ic * 512, (ic + 1) * 512)
                nc.vector.tensor_tensor(
                    out=avg[mt][0:64, isl],
                    in0=avraw[mt][0:64, ic * 512:(ic + 1) * 512],
                    in1=cb_ps[0:64, :],
                    op=ALU.mult,
                )
                nc.vector.tensor_tensor(
                    out=avg[mt][64:128, isl],
                    in0=avraw[mt][64:128, N + ic * 512:N + (ic + 1) * 512],
                    in1=cb_ps[64:128, :],
                    op=ALU.mult,
                )

            def out_block(it):
                ops = u_tile("ops")
                for mt in range(MT):
                    mm(
                        ops[:],
                        avg[mt][:, it * 128:(it + 1) * 128],
                        wo_sb[:, mt, :],
                        start=(mt == 0),
                        stop=(mt == MT - 1),
                    )
                osb = scratch.tile([128, DIM], F32, tag="osb", name="osb", bufs=3)
                nc.scalar.copy(out=osb[:], in_=ops[:])
                eng = nc.sync if it % 2 == 0 else nc.scalar
                eng.dma_start(out=out_d[it * 128:(it + 1) * 128, :], in_=osb[:])

            # ---- prologue: qk of pair 0, then gates (needed only by the
            # pair-0 gating fillers one window later) ----
            for which, ic in ((0, 0), (1, 0), (0, 1), (1, 1)):
                qk_block(0, which, ic)
            gates_block()

            # ---- pair windows ----
            QK_ORDER = ((0, 0), (1, 0), (0, 1), (1, 1))
            for mt in range(MT):
                for jc in range(JC):
                    s_block(mt, jc, 0)
                    s_block(mt, jc, 1)
                    if mt == 0:
                        v_block(jc)
                    else:
                        if jc < 4:
                            av_pass(mt - 1, jc)
                        elif jc == 4:
                            denom_dmas(mt - 1, 0)
                            denom_dmas(mt - 1, 1)
                        elif jc == 5:
                            gate_recip(mt - 1, 0)
                            gate_recip(mt - 1, 1)
                        elif jc == 6:
                            cb_avg(mt - 1, 0)
                        elif jc == 7:
                            cb_avg(mt - 1, 1)
                    if mt < MT - 1 and 2 <= jc < 6:
                        qk_block(mt + 1, *QK_ORDER[jc - 2])

            # ---- tail: pair 3 AV + gating, out-projection ----
            av_pass(MT - 1, 0, drain_act=True)
            av_pass(MT - 1, 2, drain_act=True)
            denom_dmas(MT - 1, 0)
            gate_recip(MT - 1, 0)
            av_pass(MT - 1, 1, drain_act=True)
            cb_avg(MT - 1, 0)
            av_pass(MT - 1, 3, drain_act=True)
            out_block(0)
            denom_dmas(MT - 1, 1)
            gate_recip(MT - 1, 1)
            for it in range(1, 4):
                out_block(it)
            cb_avg(MT - 1, 1)
            for it in range(4, NT):
                out_block(it)


_NC_CACHE = None


def _get_nc():
    global _NC_CACHE
    if _NC_CACHE is None:
        _NC_CACHE = build_kernel()
    return _NC_CACHE


def kernel(**inputs):
    x = np.ascontiguousarray(np.asarray(inputs["x"], dtype=np.float32))
    gamma = np.ascontiguousarray(np.asarray(inputs["gamma"], dtype=np.float32))
    w_qkv = np.ascontiguousarray(np.asarray(inputs["w_qkv"], dtype=np.float32))
    w_g = np.ascontiguousarray(np.asarray(inputs["w_g"], dtype=np.float32))
    b_g = np.ascontiguousarray(np.asarray(inputs["b_g"], dtype=np.float32))
    w_o = np.ascontiguousarray(np.asarray(inputs["w_o"], dtype=np.float32))

    nc = _get_nc()
    in_maps = []
    for i in range(N_CORES):
        in_maps.append(
            {
                "x": np.ascontiguousarray(x[i]),
                "gamma": gamma,
                "w_qkv": w_qkv,
                "w_g": w_g,
                "b_g": b_g,
                "w_o": w_o,
            }
        )
    res = run_bass_kernel_spmd(nc, in_maps, core_ids=list(range(N_CORES)))
    out = np.stack([res.results[i]["out"] for i in range(N_CORES)], axis=0)
    return out.astype(np.float32)


if __name__ == "__main__":
    rng = np.random.default_rng(0)
    ins = {
        "x": rng.standard_normal((B, N, DIM), dtype=np.float32),
        "gamma": np.ones((DIM,), np.float32),
        "w_qkv": (rng.standard_normal((DIM, 3 * DIM), dtype=np.float32) / np.sqrt(DIM)),
        "w_g": (rng.standard_normal((DIM, HEADS), dtype=np.float32) / np.sqrt(DIM)),
        "b_g": np.zeros((HEADS,), np.float32),
        "w_o": (rng.standard_normal((DIM, DIM), dtype=np.float32) / np.sqrt(DIM)),
    }
    out = kernel(**ins)
    print("out", out.shape, out.dtype, float(np.abs(out).mean()))


# revision 30
# speedup vs baseline: 1.0843x; 1.0843x over previous
"""Gated multi-head attention (RMSNorm + RoPE + SDPA + sigmoid head gates + out-proj)
as a Trainium2 Bass/Tile kernel, data-parallel over batch across 8 NeuronCores.

Problem shapes (hardcoded): b=8, n=1024, dim=512, heads=8, dim_head=64, theta=1e4.
Each core processes one batch element; no collectives needed.

v3: software-pipelined single PE stream, float32r projections.

The scalar engine's exp of the n^2 score matrix (64 tiles of [128,1024],
~80us) is the pacing resource. The kernel runs 4 pair-windows of 8 "beats"
(beat = S^T tiles for one key chunk, both heads, + their exps); all other PE
work — v/qk projections of the next pair, the previous pair's AV passes and
gate broadcast, out-projection — is emitted as beat fillers so the tensor
engine stays dense (HAM p-state) and no other engine exceeds the exp budget.

float32r: q/k/v/gate projections consume the fp32 weight stages and the fp32
transposed activations directly (1 cycle/row at >=256 free size), so there
are no weight fold-casts at all; gamma folds into the transpose drains.
Scores/AV stay bf16 (qT/kT/es/v_aug) for SBUF capacity. The AV matmul's
ones-columns produce the softmax denominator on the spare partition half.

Engine roles:
  ACT: RMS square/sqrt, half the transpose drains (Copy*gamma), gates
       exp (sigmoid via 1/(1+e^-z): stays on the Exp act table), 64x exp,
       tail out-copies
  DVE: RMS recip+scale, other drains, RoPE shuffle/mul/add, v_aug + avraw
       psum drains, denominator recip, gate scale
  Pool(gpsimd): cos/sin/wg/wo/identity SWDGE loads, w_o cast, RoPE sin mul,
       v_aug ones memset (gpsimd never touches PSUM)
  SP:  x/w_qkv/gamma loads, denominator row moves, output stores
PSUM: 2x[128,1024] score tiles (4 banks; also transpose staging) +
4x[128,512] utility tiles (4 banks) shared by v/qk/gates/AV/cb/out-proj.
"""

import sys

if "/opt/trn_rl_repo" not in sys.path:
    sys.path.insert(0, "/opt/trn_rl_repo")

import numpy as np

import concourse.bass as bass
import concourse.tile as tile
from concourse import bacc, mybir
from concourse.bass_utils import run_bass_kernel_spmd

F32 = mybir.dt.float32
F32R = mybir.dt.float32r
BF16 = mybir.dt.bfloat16
AF = mybir.ActivationFunctionType
ALU = mybir.AluOpType

B = 8
N = 1024
DIM = 512
HEADS = 8
DHEAD = 64
THETA = 10000.0
N_CORES = 8

NT = N // 128  # 8 token row tiles
KO = DIM // 128  # 4 contraction chunks
NC_ = N // 512  # 2 query column chunks of 512
MT = 4  # head pairs (2 heads x 64 dims = 128 partitions)
JC = 8  # key chunks of 128


def _rope_tables():
    """cos2T/sinS2T [128, N]: rows p = (h%2)*64 + d; identical per head half.

    sinS2T row 2t   = -sin(n * invf[t])  (multiplies shuffled value q[2t+1])
    sinS2T row 2t+1 = +sin(n * invf[t])
    """
    inv_freq = 1.0 / (THETA ** (np.arange(0, DHEAD, 2, dtype=np.float64) / DHEAD))
    pos = np.arange(N, dtype=np.float64)
    freqs = pos[None, :] * np.repeat(inv_freq, 2)[:, None]  # [64, N]
    cos = np.cos(freqs)
    sin = np.sin(freqs)
    sign = np.where(np.arange(DHEAD) % 2 == 0, -1.0, 1.0)[:, None]
    sin_signed = sin * sign
    cos2 = np.concatenate([cos, cos], axis=0).astype(np.float32)
    sin2 = np.concatenate([sin_signed, sin_signed], axis=0).astype(np.float32)
    return np.ascontiguousarray(cos2), np.ascontiguousarray(sin2)


def build_kernel():
    import ml_dtypes

    nc = bacc.Bacc("TRN2", target_bir_lowering=False, debug=False, num_devices=N_CORES)

    x_d = nc.dram_tensor("x", [N, DIM], F32, kind="ExternalInput").ap()
    gamma_d = nc.dram_tensor("gamma", [DIM], F32, kind="ExternalInput").ap()
    wqkv_d = nc.dram_tensor("w_qkv", [DIM, 3 * DIM], F32, kind="ExternalInput").ap()
    wg_d = nc.dram_tensor("w_g", [DIM, HEADS], F32, kind="ExternalInput").ap()
    bg_d = nc.dram_tensor("b_g", [HEADS], F32, kind="ExternalInput").ap()
    wo_d = nc.dram_tensor("w_o", [DIM, DIM], F32, kind="ExternalInput").ap()
    out_d = nc.dram_tensor("out", [N, DIM], F32, kind="ExternalOutput").ap()

    cos_np, sin_np = _rope_tables()
    cos_d = nc.inline_tensor(cos_np.astype(ml_dtypes.bfloat16), name="rope_cos").ap()
    sin_d = nc.inline_tensor(sin_np.astype(ml_dtypes.bfloat16), name="rope_sin").ap()
    eye_d = nc.inline_tensor(np.eye(128, dtype=np.float32), name="eye128").ap()

    # E[h, mt*128 + p] = 1 if head h owns partition p of pair-tile mt
    e_np = np.zeros((HEADS, MT * 128), np.float32)
    for mt in range(MT):
        for p in range(128):
            e_np[2 * mt + p // 64, mt * 128 + p] = 1.0
    e_d = nc.inline_tensor(e_np.astype(ml_dtypes.bfloat16), name="gate_bcast_e").ap()

    with tile.TileContext(nc) as tc:
        _build_tile(
            nc, tc, x_d, gamma_d, wqkv_d, wg_d, bg_d, wo_d, cos_d, sin_d, eye_d, e_d, out_d
        )

    nc.compile()
    return nc


def _build_tile(
    nc, tc, x_d, gamma_d, wqkv_d, wg_d, bg_d, wo_d, cos_d, sin_d, eye_d, e_d, out_d
):
    from contextlib import ExitStack

    ctx = ExitStack()
    with ctx:
        singles = ctx.enter_context(tc.tile_pool(name="singles", bufs=1))
        wpool = ctx.enter_context(tc.tile_pool(name="weights", bufs=1))
        wstage = ctx.enter_context(tc.tile_pool(name="wstage", bufs=1))
        xpool = ctx.enter_context(tc.tile_pool(name="x", bufs=1))
        xtbp = ctx.enter_context(tc.tile_pool(name="xtb", bufs=1))
        xtp = ctx.enter_context(tc.tile_pool(name="xhatT", bufs=1))
        qkpool = ctx.enter_context(tc.tile_pool(name="qk", bufs=1))
        vpool = ctx.enter_context(tc.tile_pool(name="vaug", bufs=1))
        spool = ctx.enter_context(tc.tile_pool(name="expS", bufs=24))
        gpool = ctx.enter_context(tc.tile_pool(name="gates", bufs=1))
        avpool = ctx.enter_context(tc.tile_pool(name="avg", bufs=1))
        scratch = ctx.enter_context(tc.tile_pool(name="scratch", bufs=2))

        # ---- x + gamma + w_qkv on the SP ring ----
        xts = [
            xpool.tile([128, DIM], F32, tag=f"xt{it % 6}", name=f"xt{it}")
            for it in range(NT)
        ]
        for it in (0, 1, 2, 3):
            nc.sync.dma_start(out=xts[it][:], in_=x_d[it * 128:(it + 1) * 128, :])
        gamma_sb = singles.tile([128, KO], F32)
        nc.sync.dma_start(
            out=gamma_sb[:], in_=gamma_d.rearrange("(ko ki) -> ki ko", ki=128)
        )
        bg_sb = singles.tile([HEADS, 1], F32)
        nc.sync.dma_start(out=bg_sb[:], in_=bg_d.rearrange("(h o) -> h o", o=1))
        def _w_stage(ko):
            return wstage.tile(
                [128, 3 * DIM], F32, tag=f"ws{ko % 2}", name=f"ws{ko}", bufs=1
            )

        def _w_casts(ko, ws):
            # q|k columns: ACT/DVE right after the RMS stream drains;
            # v columns: Pool (first needed a window later)
            if ko % 2 == 0:
                nc.scalar.copy(out=wqkv_bf[:, ko, 0:2 * DIM], in_=ws[:, 0:2 * DIM])
            else:
                nc.vector.tensor_copy(
                    out=wqkv_bf[:, ko, 0:2 * DIM], in_=ws[:, 0:2 * DIM]
                )

        def _w_casts_v(ko, ws):
            nc.gpsimd.tensor_copy(
                out=wqkv_bf[:, ko, 2 * DIM:3 * DIM], in_=ws[:, 2 * DIM:3 * DIM]
            )

        wss = [_w_stage(0), _w_stage(1)]
        nc.sync.dma_start(out=wss[0][:], in_=wqkv_d[0:128, :])
        nc.sync.dma_start(out=wss[1][:], in_=wqkv_d[128:256, :])
        for it in (4, 5, 6, 7):
            nc.sync.dma_start(out=xts[it][:], in_=x_d[it * 128:(it + 1) * 128, :])
        # Preload the Sqrt act-table during DMA dead time: Square (every
        # table) then the real Sqrts would otherwise force a reload mid-RMS.
        warmt = scratch.tile([1, 1], F32, tag="warmt", name="warmt", bufs=1)
        nc.gpsimd.memset(warmt[:], 1.0)
        nc.scalar.activation(out=warmt[:], in_=warmt[:], func=AF.Sqrt)

        # ---- small/constant loads on the Pool SWDGE ring ----
        ident = singles.tile([128, 128], BF16)
        nc.gpsimd.dma_start(out=ident[:], in_=eye_d[:])
        cosT = singles.tile([128, N], BF16)
        sinT = singles.tile([128, N], BF16)
        nc.gpsimd.dma_start(out=cosT[:], in_=cos_d[:])
        nc.gpsimd.dma_start(out=sinT[:], in_=sin_d[:])
        wg_s = wpool.tile([128, KO, HEADS], F32)
        nc.gpsimd.dma_start(
            out=wg_s[:], in_=wg_d.rearrange("(ko ki) h -> ki ko h", ki=128)
        )
        wg_bf = wpool.tile([128, KO, HEADS], BF16)
        nc.gpsimd.tensor_copy(out=wg_bf[:], in_=wg_s[:])
        e_sb = singles.tile([HEADS, MT * 128], BF16, name="e_sb")
        nc.gpsimd.dma_start(out=e_sb[:], in_=e_d[:])



        # ---- RMSnorm -> xtb (f32), 1-deep pipelined on ACT ----
        xhatT = [
            xtp.tile([128, N], BF16, tag=f"xhatT{ko}", name=f"xhatT{ko}")
            for ko in range(KO)
        ]
        xtb = [
            xtbp.tile([128, DIM], BF16, tag=f"xtb{it}", name=f"xtb{it}")
            for it in range(NT)
        ]
        sss = []

        def _rms_head(it):
            ss = scratch.tile([128, 1], F32, tag=f"ss{it}", name=f"ss{it}", bufs=1)
            # Square's elementwise output is unused; scribble it into xtb[it],
            # which the tensor_scalar_mul in _rms_tail overwrites.
            nc.scalar.activation(
                out=xtb[it][:], in_=xts[it][:], func=AF.Square, accum_out=ss[:]
            )
            sss.append(ss)

        def _rms_tail(it):
            nc.scalar.activation(
                out=sss[it][:], in_=sss[it][:], func=AF.Sqrt, scale=1.0 / DIM
            )
            nc.vector.reciprocal(out=sss[it][:], in_=sss[it][:])
            nc.vector.tensor_scalar_mul(
                out=xtb[it][:], in0=xts[it][:], scalar1=sss[it][:]
            )

        _rms_head(0)
        for it in range(1, NT):
            _rms_head(it)
            _rms_tail(it - 1)
        _rms_tail(NT - 1)

        negbg_sb = singles.tile([HEADS, 1], F32)
        nc.vector.tensor_scalar_mul(out=negbg_sb[:], in0=bg_sb[:], scalar1=-1.0)
        wqkv_bf = wpool.tile([128, KO, 3 * DIM], BF16)
        _w_casts(0, wss[0])
        _w_casts(1, wss[1])
        _w_casts_v(0, wss[0])
        _w_casts_v(1, wss[1])
        wss.append(_w_stage(2))
        nc.sync.dma_start(out=wss[2][:], in_=wqkv_d[256:384, :])
        _w_casts(2, wss[2])
        wss.append(_w_stage(3))
        nc.sync.dma_start(out=wss[3][:], in_=wqkv_d[384:512, :])
        _w_casts(3, wss[3])
        _w_casts_v(2, wss[2])
        _w_casts_v(3, wss[3])

        # ---- persistent SBUF for attention ----
        v_aug = [
            vpool.tile([128, HEADS * 128], BF16, tag=f"va{it}", name=f"va{it}")
            for it in range(NT)
        ]
        for it in range(NT):
            va3 = v_aug[it][:].rearrange("p (q c) -> p q c", q=HEADS // 2)
            nc.gpsimd.memset(va3[:, :, 64:192], 1.0)

        wo_st = wpool.tile([128, KO, DIM], F32)
        nc.gpsimd.dma_start(
            out=wo_st[:], in_=wo_d.rearrange("(ko ki) d -> ki ko d", ki=128)
        )
        wo_sb = wpool.tile([128, KO, DIM], BF16)
        for ko in range(KO):
            nc.gpsimd.tensor_copy(out=wo_sb[:, ko, :], in_=wo_st[:, ko, :])
        qT = [qkpool.tile([128, N], BF16, tag=f"q{mt}", name=f"q{mt}") for mt in range(MT)]
        kT = [qkpool.tile([128, N], BF16, tag=f"k{mt}", name=f"k{mt}") for mt in range(MT)]
        avg = [
            avpool.tile([128, N], BF16, tag=f"avg{mt}", name=f"avg{mt}")
            for mt in range(MT)
        ]
        gT = gpool.tile([HEADS, N], F32)
        denomW = gpool.tile([HEADS, N], F32)
        nc.gpsimd.memset(denomW[:], 1.0)
        cT = gpool.tile([HEADS, N], BF16)

        shuf_mask = [(i ^ 1) for i in range(32)]
        scale = 1.0 / float(np.sqrt(DHEAD))

        es = [[[None for _ in range(2)] for _ in range(JC)] for _ in range(MT)]
        avraw = {}

        with (
            tc.tile_pool(name="ps_s", bufs=2, space="PSUM") as ps_s,
            tc.tile_pool(name="ps_u", bufs=4, space="PSUM") as ps_u,
        ):
            def mm(out_ps, lhsT, rhs, start, stop):
                nc.tensor.matmul(out_ps, lhsT, rhs, start=start, stop=stop)

            def u_tile(nm):
                return ps_u.tile([128, 512], F32, tag="u", name=nm)

            # ---- PE transpose of xtb -> xhatT through the S psum banks
            # (bf16 view of the f32 tiles); gamma folds into the drains ----
            for ic in range(NC_):
                trp = ps_s.tile([128, N], F32, tag="sps", name=f"tr{ic}")
                trv = trp[:].bitcast(BF16).rearrange("p (ko c) -> p ko c", ko=KO)
                for s in range(4):
                    it = ic * 4 + s
                    for ko in range(KO):
                        nc.tensor.transpose(
                            trv[:, ko, s * 128:(s + 1) * 128],
                            xtb[it][:, ko * 128:(ko + 1) * 128],
                            ident[:],
                        )
                for ko in range(KO):
                    dst = xhatT[ko][:, ic * 512:(ic + 1) * 512]
                    if ko % 2 == 0:
                        nc.scalar.activation(
                            out=dst,
                            in_=trv[:, ko, :],
                            func=AF.Copy,
                            scale=gamma_sb[:, ko:ko + 1],
                        )
                    else:
                        nc.vector.tensor_scalar_mul(
                            out=dst,
                            in0=trv[:, ko, :],
                            scalar1=gamma_sb[:, ko:ko + 1],
                        )

            def gates_block():
                # sigmoid(z) = 1/(1+exp(-z)) — keeps ACT on the Exp table set
                for ic in range(NC_):
                    g_ps = u_tile("gps")
                    for ko in range(KO):
                        mm(
                            g_ps[0:HEADS, :],
                            wg_bf[:, ko, :],
                            xhatT[ko][:, ic * 512:(ic + 1) * 512],
                            start=(ko == 0),
                            stop=(ko == KO - 1),
                        )
                    isl = slice(ic * 512, (ic + 1) * 512)
                    ge = scratch.tile([HEADS, 512], F32, tag="ge", name="ge", bufs=1)
                    nc.scalar.activation(
                        out=ge[:],
                        in_=g_ps[0:HEADS, :],
                        func=AF.Exp,
                        scale=-1.0,
                        bias=negbg_sb[:],
                    )
                    nc.vector.tensor_scalar_add(out=ge[:], in0=ge[:], scalar1=1.0)
                    nc.vector.reciprocal_approx_fast(out=gT[:, isl], in_=ge[:])

            def v_block(it):
                """Project v row-tile it; drain into v_aug pair blocks."""
                vps = u_tile("vps")
                for ko in range(KO):
                    mm(
                        vps[:],
                        xhatT[ko][:, it * 128:(it + 1) * 128],
                        wqkv_bf[:, ko, 2 * DIM:3 * DIM],
                        start=(ko == 0),
                        stop=(ko == KO - 1),
                    )
                # src col q*128+s*64+d -> dst col q*256 + s*192 + d
                vsrc = vps[:].rearrange("p (q s d) -> p q s d", q=4, s=2)
                vdst = v_aug[it][:].rearrange("p (q c) -> p q c", q=4)
                nc.vector.tensor_copy(out=vdst[:, :, 0:64], in_=vsrc[:, :, 0, :])
                nc.vector.tensor_copy(out=vdst[:, :, 192:256], in_=vsrc[:, :, 1, :])

            def qk_block(mt, which, ic):
                """Project+RoPE one [128, 512] chunk of qT/kT for pair mt."""
                dest = qT if which == 0 else kT
                col0 = which * DIM + mt * 128
                nslice = slice(ic * 512, (ic + 1) * 512)
                pps = u_tile("pps")
                for ko in range(KO):
                    mm(
                        pps[:],
                        wqkv_bf[:, ko, col0:col0 + 128],
                        xhatT[ko][:, nslice],
                        start=(ko == 0),
                        stop=(ko == KO - 1),
                    )
                shuf = scratch.tile([128, 512], F32, tag="shuf", name="shuf", bufs=2)
                nc.vector.stream_shuffle(shuf[:], pps[:], mask=shuf_mask)
                t1 = scratch.tile([128, 512], F32, tag="rt1", name="rt1", bufs=2)
                nc.vector.tensor_tensor(
                    out=t1[:], in0=pps[:], in1=cosT[:, nslice], op=ALU.mult
                )
                t2 = scratch.tile([128, 512], F32, tag="rt2", name="rt2", bufs=2)
                nc.gpsimd.tensor_tensor(
                    out=t2[:], in0=shuf[:], in1=sinT[:, nslice], op=ALU.mult
                )
                nc.vector.tensor_tensor(
                    out=dest[mt][:, nslice], in0=t1[:], in1=t2[:], op=ALU.add
                )

            def s_block(mt, jc, hh):
                """Scores S^T [128 keys, N queries] for (pair, key chunk, head) + exp."""
                pr = slice(hh * 64, (hh + 1) * 64)
                sp = ps_s.tile([128, N], F32, tag="sps", name="sps")
                for ic in range(NC_):
                    mm(
                        sp[:, ic * 512:(ic + 1) * 512],
                        kT[mt][pr, jc * 128:(jc + 1) * 128],
                        qT[mt][pr, ic * 512:(ic + 1) * 512],
                        start=True,
                        stop=True,
                    )
                e = spool.tile([128, N], BF16, tag="es", name="es")
                nc.scalar.activation(out=e[:], in_=sp[:], func=AF.Exp, scale=scale)
                es[mt][jc][hh] = e

            AV_ORDER = ((0, 0), (1, 0), (0, 1), (1, 1))  # (ic, hh)

            def av_pass(mt, p, drain_act=False):
                """One AV accumulation pass (ic, hh) over all key chunks."""
                ic, hh = AV_ORDER[p]
                h = 2 * mt + hh
                base = (h // 2) * 256 + (h % 2) * 128
                if mt not in avraw:
                    avraw[mt] = avpool.tile(
                        [128, 2 * N], F32, tag="avraw", name=f"avraw{mt}", bufs=2
                    )
                avp = u_tile(f"avp{p}")
                for jc in range(JC):
                    mm(
                        avp[:],
                        v_aug[jc][:, base:base + 128],
                        es[mt][jc][hh][:, ic * 512:(ic + 1) * 512],
                        start=(jc == 0),
                        stop=(jc == JC - 1),
                    )
                dst = avraw[mt][:, hh * N + ic * 512:hh * N + (ic + 1) * 512]
                if drain_act:
                    nc.scalar.copy(out=dst, in_=avp[:])
                else:
                    nc.vector.tensor_copy(out=dst, in_=avp[:])

            def denom_dmas(mt, ic):
                h0 = 2 * mt
                nc.sync.dma_start(
                    out=denomW[h0:h0 + 1, ic * 512:(ic + 1) * 512],
                    in_=avraw[mt][64:65, ic * 512:(ic + 1) * 512],
                )
                nc.sync.dma_start(
                    out=denomW[h0 + 1:h0 + 2, ic * 512:(ic + 1) * 512],
                    in_=avraw[mt][0:1, N + ic * 512:N + (ic + 1) * 512],
                )

            def gate_recip(mt, ic):
                # DVE partition bases must be 32-aligned: process all 8 head
                # rows (cost is free-size-bound); stale rows are finite (the
                # denomW memset) and meet zero E-columns in cb_avg.
                isl = slice(ic * 512, (ic + 1) * 512)
                dscr = scratch.tile([HEADS, 512], F32, tag="dscr", name="dscr", bufs=1)
                nc.vector.reciprocal_approx_fast(out=dscr[:], in_=denomW[:, isl])
                nc.vector.tensor_tensor(
                    out=cT[:, isl], in0=dscr[:], in1=gT[:, isl], op=ALU.mult
                )

            def cb_avg(mt, ic):
                """Broadcast gate/denominator row to pair partitions; scale AV."""
                cb_ps = u_tile("cbps")
                mm(
                    cb_ps[:],
                    e_sb[:, mt * 128:(mt + 1) * 128],
                    cT[:, ic * 512:(ic + 1) * 512],
                    start=True,
                    stop=True,
                )
                isl = slice(ic * 512, (ic + 1) * 512)
                nc.vector.tensor_tensor(
                    out=avg[mt][0:64, isl],
                    in0=avraw[mt][0:64, ic * 512:(ic + 1) * 512],
                    in1=cb_ps[0:64, :],
                    op=ALU.mult,
                )
                nc.vector.tensor_tensor(
                    out=avg[mt][64:128, isl],
                    in0=avraw[mt][64:128, N + ic * 512:N + (ic + 1) * 512],
                    in1=cb_ps[64:128, :],
                    op=ALU.mult,
                )

            def out_block(it):
                ops = u_tile("ops")
                for mt in range(MT):
                    mm(
                        ops[:],
                        avg[mt][:, it * 128:(it + 1) * 128],
                        wo_sb[:, mt, :],
                        start=(mt == 0),
                        stop=(mt == MT - 1),
                    )
                osb = scratch.tile([128, DIM], F32, tag="osb", name="osb", bufs=3)
                nc.scalar.copy(out=osb[:], in_=ops[:])
                eng = nc.sync if it % 2 == 0 else nc.scalar
                eng.dma_start(out=out_d[it * 128:(it + 1) * 128, :], in_=osb[:])

            # ---- prologue: qk of pair 0, then gates (needed only by the
            # pair-0 gating fillers one window later) ----
            for which, ic in ((0, 0), (1, 0), (0, 1), (1, 1)):
                qk_block(0, which, ic)
            gates_block()

            # ---- pair windows ----
            QK_ORDER = ((0, 0), (1, 0), (0, 1), (1, 1))
            for mt in range(MT):
                for jc in range(JC):
                    s_block(mt, jc, 0)
                    s_block(mt, jc, 1)
                    if mt == 0:
                        v_block(jc)
                    else:
                        if jc < 4:
                            av_pass(mt - 1, jc)
                        elif jc == 4:
                            denom_dmas(mt - 1, 0)
                            denom_dmas(mt - 1, 1)
                        elif jc == 5:
                            gate_recip(mt - 1, 0)
                            gate_recip(mt - 1, 1)
                        elif jc == 6:
                            cb_avg(mt - 1, 0)
                        elif jc == 7:
                            cb_avg(mt - 1, 1)
                    if mt < MT - 1 and 2 <= jc < 6:
                        qk_block(mt + 1, *QK_ORDER[jc - 2])

            # ---- tail: pair 3 AV + gating, out-projection ----
            av_pass(MT - 1, 0, drain_act=True)
            av_pass(MT - 1, 2, drain_act=True)
            denom_dmas(MT - 1, 0)
            gate_recip(MT - 1, 0)
            av_pass(MT - 1, 1, drain_act=True)
            cb_avg(MT - 1, 0)
            av_pass(MT - 1, 3, drain_act=True)
            out_block(0)
            denom_dmas(MT - 1, 1)
            gate_recip(MT - 1, 1)
            for it in range(1, 4):
                out_block(it)
            cb_avg(MT - 1, 1)
            for it in range(4, NT):
                out_block(it)


_NC_CACHE = None


def _get_nc():
    global _NC_CACHE
    if _NC_CACHE is None:
        _NC_CACHE = build_kernel()
    return _NC_CACHE


def kernel(**inputs):
    x = np.ascontiguousarray(np.asarray(inputs["x"], dtype=np.float32))
    gamma = np.ascontiguousarray(np.asarray(inputs["gamma"], dtype=np.float32))
    w_qkv = np.ascontiguousarray(np.asarray(inputs["w_qkv"], dtype=np.float32))
    w_g = np.ascontiguousarray(np.asarray(inputs["w_g"], dtype=np.float32))
    b_g = np.ascontiguousarray(np.asarray(inputs["b_g"], dtype=np.float32))
    w_o = np.ascontiguousarray(np.asarray(inputs["w_o"], dtype=np.float32))

    nc = _get_nc()
    in_maps = []
    for i in range(N_CORES):
        in_maps.append(
            {
                "x": np.ascontiguousarray(x[i]),
                "gamma": gamma,
                "w_qkv": w_qkv,
                "w_g": w_g,
                "b_g": b_g,
                "w_o": w_o,
            }
        )
    res = run_bass_kernel_spmd(nc, in_maps, core_ids=list(range(N_CORES)))
    out = np.stack([res.results[i]["out"] for i in range(N_CORES)], axis=0)
    return out.astype(np.float32)


if __name__ == "__main__":
    rng = np.random.default_rng(0)
    ins = {
        "x": rng.standard_normal((B, N, DIM), dtype=np.float32),
        "gamma": np.ones((DIM,), np.float32),
        "w_qkv": (rng.standard_normal((DIM, 3 * DIM), dtype=np.float32) / np.sqrt(DIM)),
        "w_g": (rng.standard_normal((DIM, HEADS), dtype=np.float32) / np.sqrt(DIM)),
        "b_g": np.zeros((HEADS,), np.float32),
        "w_o": (rng.standard_normal((DIM, DIM), dtype=np.float32) / np.sqrt(DIM)),
    }
    out = kernel(**ins)
    print("out", out.shape, out.dtype, float(np.abs(out).mean()))
